# revision 4
# baseline (speedup 1.0000x reference)
"""Trainium2 Bass kernel for nn_LocalDownsample (segment mean-pool).

Contract: kernel(**inputs) takes FULL inputs (x [8,4096,512] f32,
regions [8,4096] i64, max_n=512), returns FULL output [8,512,512] f32.

Sharding: pure data parallel - batch b -> core b. Per core:
  out[n-1, :] = mean over tokens t with regions[t] == n of x[t, :]   (0 if empty)

Algorithm (v2, sorted-token single-window matmul):
  Segment mean is invariant to token order, so the host sorts each batch's
  tokens by region id. Tokens whose regions fall in output window
  m = [128m+1, 128(m+1)] form a contiguous run; each run is padded to a
  multiple of 128 tokens (pad slots get region id 0 / scale 0), giving
  tiles_m 128-token tiles per window, identical across cores (max over
  cores) so one SPMD program serves all 8.

  The host also folds the mean into the tokens: scale s[t] = 1/count[region[t]]
  (counts via bincount), and converts x to fp16 laid out [128, J, 512]
  (tile j holds sorted tokens [128j, 128j+128) across partitions).

  Per core the device then does, per tile j in window m:
    oh_j [128,128] f16 = (iota[:, 128m:128m+128] == r[:, j]) * s[:, j]
        -- one fused DVE tensor_scalar (is_equal then mult)
    acc[m] [128,512] f32 PSUM += oh_j.T @ x16[:, j, :]   -- one matmul
  i.e. ~36 matmuls total instead of the 256 of the dense one-hot approach.
  After a window's last tile: ACT copies acc[m] -> f16 SBUF, DMA out.
  Output returned as f16 -> f32 on host.

  x16 streams on the SP HWDGE ring in ramped chunks so the PE starts early;
  r/s ride the ACT ring.  Bottleneck is the ~4.5 MiB x16 DMA (~13 us at
  ~358 GB/s/core); PE (~8 us) and DVE (~3.5 us) hide under it.
"""

import numpy as np

import concourse.bacc as bacc
import concourse.bass as bass  # noqa: F401
import concourse.mybir as mybir
import concourse.tile as tile
from concourse.bass_utils import run_bass_kernel_spmd

P = 128          # SBUF partitions
T = 4096         # tokens per batch
C = 512          # channels
NR = 512         # number of regions (max_n)
MC = NR // P     # 4 output row chunks (windows)
NCORES = 8

F16 = mybir.dt.float16
F32 = mybir.dt.float32

_CACHE = {}


def _chunks_for(total):
    """Ramped DMA chunk sizes (in tiles) summing to `total`."""
    ramp = [1, 1, 2, 4]
    out = []
    left = total
    for r in ramp:
        if left <= 0:
            break
        c = min(r, left)
        out.append(c)
        left -= c
    while left > 0:
        c = min(8, left)
        out.append(c)
        left -= c
    return tuple(out)


def _build(tiles, repeats=1, out16=True):
    """tiles: per-window tile counts, e.g. (9, 9, 9, 9)."""
    TOTJ = sum(tiles)
    CHUNKS = _chunks_for(TOTJ)
    # window index of each tile j
    win = []
    for m, tm in enumerate(tiles):
        win += [m] * tm
    first = {}
    last = {}
    for j, m in enumerate(win):
        first.setdefault(m, j)
        last[m] = j

    OD = F16 if out16 else F32

    nc = bacc.Bacc(None, target_bir_lowering=False)
    x_d = nc.dram_tensor("xall", [P, TOTJ, C], F16, kind="ExternalInput")
    rs_d = nc.dram_tensor("rs", [P, 2 * TOTJ], F32, kind="ExternalInput")
    o_d = nc.dram_tensor("out", [NR, C], OD, kind="ExternalOutput")

    with tile.TileContext(nc) as tc:
        with (
            tc.tile_pool(name="const", bufs=1) as cpool,
            tc.tile_pool(name="inp", bufs=2) as inp_pool,
            tc.tile_pool(name="xf", bufs=len(CHUNKS)) as xf_pool,
            tc.tile_pool(name="oh", bufs=TOTJ) as oh_pool,
            tc.tile_pool(name="eplg", bufs=2) as out_pool,
            tc.tile_pool(name="psum", bufs=1, space="PSUM") as psum_pool,
        ):
            iota16 = cpool.tile([P, NR], F16, tag="iota16")
            nc.gpsimd.iota(
                iota16[:], pattern=[[1, NR]], base=1, channel_multiplier=0,
                allow_small_or_imprecise_dtypes=True,  # 1..512 exact in fp16
            )

            def body():
                # r/s on the ACT ring; x owns the SP ring from t=0
                rs_t = inp_pool.tile([P, 2 * TOTJ], F32, tag="rs")
                nc.scalar.dma_start(rs_t[:], rs_d[:, :])

                xf = []          # per tile j: (chunk_tile, index within chunk)
                j0 = 0
                for ci, csz in enumerate(CHUNKS):
                    t = xf_pool.tile([P, csz, C], F16, name=f"xfc{ci}", tag="xf")
                    nc.sync.dma_start(t[:], x_d[:, j0 : j0 + csz, :])
                    for jj in range(csz):
                        xf.append((t, jj))
                    j0 += csz

                acc = [
                    psum_pool.tile([P, C], F32, name=f"acc{m}", tag=f"acc{m}")
                    for m in range(MC)
                ]

                for j in range(TOTJ):
                    m = win[j]
                    oh = oh_pool.tile([P, P], F16, name=f"oh{j}", tag="oh")
                    nc.vector.tensor_scalar(
                        out=oh[:],
                        in0=iota16[:, m * P : (m + 1) * P],
                        scalar1=rs_t[:, j : j + 1],
                        scalar2=rs_t[:, TOTJ + j : TOTJ + j + 1],
                        op0=mybir.AluOpType.is_equal,
                        op1=mybir.AluOpType.mult,
                    )
                    xt, jj = xf[j]
                    nc.tensor.matmul(
                        acc[m][:],
                        lhsT=oh[:],
                        rhs=xt[:, jj, :],
                        start=(j == first[m]),
                        stop=(j == last[m]),
                        skip_group_check=True,
                    )
                    if j == last[m]:
                        osb = out_pool.tile([P, C], OD, name=f"osb{m}", tag=f"osb{m}")
                        nc.scalar.copy(osb[:], acc[m][:])
                        nc.scalar.dma_start(o_d[m * P : (m + 1) * P, :], osb[:])

                for m in range(MC):
                    if tiles[m] == 0:
                        osb = out_pool.tile([P, C], OD, name=f"osbz{m}", tag=f"osb{m}")
                        nc.vector.memset(osb[:], 0.0)
                        nc.scalar.dma_start(o_d[m * P : (m + 1) * P, :], osb[:])

            if repeats == 1:
                body()
            else:
                with tc.For_i(0, repeats, 1, hint_engines=(mybir.EngineType.PE,)):
                    body()

    nc.compile()
    return nc


def _get_nc(tiles, **cfg):
    key = (tuple(tiles), tuple(sorted(cfg.items())))
    if key not in _CACHE:
        _CACHE[key] = _build(tuple(tiles), **cfg)
    return _CACHE[key]


def prepare(x, regions):
    """Sort/pad/scale on host. Returns (tiles, in_maps)."""
    x = np.asarray(x, dtype=np.float32)
    regions = np.asarray(regions).astype(np.int64)
    B = x.shape[0]

    sorted_r = []
    sorted_x16 = []
    cnt_w = np.zeros((B, MC), dtype=np.int64)
    for b in range(B):
        order = np.argsort(regions[b], kind="stable")
        rs = regions[b][order]
        sorted_r.append(rs)
        sorted_x16.append(x[b][order].astype(np.float16))
        # tokens per 128-region window
        bounds = np.searchsorted(rs, np.arange(MC + 1) * P + 1)
        cnt_w[b] = np.diff(bounds)

    tiles = tuple(int(v) for v in np.ceil(cnt_w.max(axis=0) / P).astype(np.int64))
    TOTJ = sum(tiles)

    in_maps = []
    for b in range(B):
        rs = sorted_r[b]
        cnt = np.bincount(rs, minlength=NR + 1)  # index = region id
        s_tok = (1.0 / cnt[rs]).astype(np.float32)
        r_tok = rs.astype(np.float32)

        xpad = np.zeros((TOTJ * P, C), dtype=np.float16)
        rpad = np.zeros(TOTJ * P, dtype=np.float32)
        spad = np.zeros(TOTJ * P, dtype=np.float32)
        off = 0
        tile_off = 0
        for m in range(MC):
            n = int(cnt_w[b][m])
            d0 = tile_off * P
            xpad[d0 : d0 + n] = sorted_x16[b][off : off + n]
            rpad[d0 : d0 + n] = r_tok[off : off + n]
            spad[d0 : d0 + n] = s_tok[off : off + n]
            off += n
            tile_off += tiles[m]

        # device layout: [partition, tile, ...] with tile j = tokens [128j, 128j+128)
        xdev = np.ascontiguousarray(xpad.reshape(TOTJ, P, C).transpose(1, 0, 2))
        rdev = rpad.reshape(TOTJ, P).T
        sdev = spad.reshape(TOTJ, P).T
        rsdev = np.ascontiguousarray(np.concatenate([rdev, sdev], axis=1))
        in_maps.append({"xall": xdev, "rs": rsdev})

    return tiles, in_maps


def kernel(x, regions, max_n, _trace=False, _tmpdir=None, _cfg=None):
    x = np.asarray(x, dtype=np.float32)
    regions = np.asarray(regions)
    assert x.shape == (NCORES, T, C), x.shape
    assert regions.shape == (NCORES, T), regions.shape
    assert int(np.asarray(max_n)) == NR

    tiles, in_maps = prepare(x, regions)
    cfg = dict(_cfg or {})
    out16 = cfg.get("out16", True)
    nc = _get_nc(tiles, **cfg)
    try:
        res = run_bass_kernel_spmd(
            nc, in_maps, core_ids=list(range(NCORES)), trace=_trace, tmpdir=_tmpdir
        )
    except Exception:
        # one retry for transient runtime/tunnel failures
        res = run_bass_kernel_spmd(
            nc, in_maps, core_ids=list(range(NCORES)), trace=_trace, tmpdir=_tmpdir
        )
    out = np.stack(
        [res.results[b]["out"].astype(np.float32) for b in range(NCORES)], axis=0
    )
    if _trace:
        kernel._last_results = res
    return out


# revision 11
# speedup vs baseline: 1.0772x; 1.0772x over previous
"""Trainium2 Bass kernel for nn_LocalDownsample (segment mean-pool).

Contract: kernel(**inputs) takes FULL inputs (x [8,4096,512] f32,
regions [8,4096] i64, max_n=512), returns FULL output [8,512,512] f32.

Sharding: pure data parallel - batch b -> core b. Per core:
  out[n-1, :] = mean over tokens t with regions[t] == n of x[t, :]   (0 if empty)

Algorithm (v3, sorted dense-stream single/dual-window matmul):
  Segment mean is invariant to token order, so the host sorts each batch's
  tokens by region id and streams them densely: tile j = sorted tokens
  [128j, 128j+128), 32 tiles, no padding. The host folds the mean into the
  tokens (scale s[t] = 1/count[region[t]], counts via bincount) and converts
  x to fp16 laid out [128, 32, 512].

  Output rows live in 4 windows of 128 regions (PSUM bank each). Most tiles'
  tokens fall entirely inside one window; tiles that may straddle a window
  boundary on ANY core (known per input, baked into the compiled program as
  the per-tile window range) issue one matmul per touched window. Per core,
  per tile j and window m in range(j):
    oh [128,128] f16 = (iota[:, 128m:128m+128] == r[:, j]) * s[:, j]
        -- one fused DVE tensor_scalar (is_equal then mult); tokens outside
           the window give zero rows, so straddle handling is uniform SPMD
    acc[m] [128,512] f32 PSUM += oh.T @ x16[:, j, :]
  ~38 matmuls total (vs 256 for the dense one-hot baseline). After a
  window's last matmul: ACT copies acc[m] -> f16 SBUF, DMA out (last window
  split in half across ACT+DVE with two parallel store DMAs to shorten the
  tail). Output returned as f16 -> f32 on host.

  r/s ride first on the SP HWDGE ring, then x16 streams in chunks ramped up
  at the start (so the PE starts early) and down at the end (so the last
  matmuls closely trail the last bytes). Bottleneck is the 4 MiB x16 DMA
  (~12 us at ~360 GB/s/core); PE (~8 us) and DVE (~3.5 us) hide under it.
"""

import numpy as np

import concourse.bacc as bacc
import concourse.bass as bass  # noqa: F401
import concourse.mybir as mybir
import concourse.tile as tile
from concourse.bass_utils import run_bass_kernel_spmd

P = 128          # SBUF partitions
T = 4096         # tokens per batch
C = 512          # channels
NR = 512         # number of regions (max_n)
MC = NR // P     # 4 output row windows
JT = T // P      # 32 tiles
NCORES = 8

F16 = mybir.dt.float16
F32 = mybir.dt.float32

_CACHE = {}

# HW-tuned default config (A/B'd on device): 6-tile mid chunks, 32 warm-up
# matmuls, r/s via SWDGE
DEFAULT_CFG = dict(mid_sz=6, warm=32, swdge_rs=True)


def _chunks_for(total, mid_sz=8):
    """DMA chunk sizes (in tiles): ramp up at the start, down at the end."""
    head = [1, 1, 2, 4]
    tail = [4, 2, 1, 1]
    mid = total - sum(head) - sum(tail)
    assert mid >= 0
    out = list(head)
    while mid > 0:
        c = min(mid_sz, mid)
        out.append(c)
        mid -= c
    return tuple(out + tail)


def _build(wsets, repeats=1, mid_sz=8, warm=0, warm_n=128, ring_alt=False,
           swdge_head=False, c0_first=False, tail_split=False, swdge_rs=False,
           one_store=False, hints="pe"):
    """wsets: per-tile (lo, hi) inclusive window range, len JT."""
    CHUNKS = _chunks_for(JT, mid_sz)
    # (j, m) matmul list in issue order; first/last per window
    mms = []
    for j, (lo, hi) in enumerate(wsets):
        for m in range(lo, hi + 1):
            mms.append((j, m))
    first = {}
    last = {}
    for j, m in mms:
        first.setdefault(m, (j, m))
        last[m] = (j, m)
    last_window = max(m for _, m in mms)

    nc = bacc.Bacc(None, target_bir_lowering=False)
    x_d = nc.dram_tensor("xall", [P, JT, C], F16, kind="ExternalInput")
    rs_d = nc.dram_tensor("rs", [P, 2 * JT], F32, kind="ExternalInput")
    o_d = nc.dram_tensor("out", [NR, C], F16, kind="ExternalOutput")

    with tile.TileContext(nc) as tc:
        with (
            tc.tile_pool(name="const", bufs=1) as cpool,
            tc.tile_pool(name="inp", bufs=2) as inp_pool,
            tc.tile_pool(name="xf", bufs=len(CHUNKS)) as xf_pool,
            tc.tile_pool(name="oh", bufs=len(mms)) as oh_pool,
            tc.tile_pool(name="eplg", bufs=2) as out_pool,
            tc.tile_pool(name="psum", bufs=1, space="PSUM") as psum_pool,
        ):
            iota16 = cpool.tile([P, NR], F16, tag="iota16")
            nc.gpsimd.iota(
                iota16[:], pattern=[[1, NR]], base=1, channel_multiplier=0,
                allow_small_or_imprecise_dtypes=True,  # 1..512 exact in fp16
            )

            def body():
                # r/s lead the ring so the first oh can build ASAP; x chunks
                # follow FIFO. swdge_head routes the latency-critical head
                # transfers through the GPSIMD SWDGE (shorter first-byte
                # latency); c0_first puts x tile 0 ahead of r/s.
                head_eng = nc.gpsimd if (swdge_head or swdge_rs) else nc.sync
                rs_t = inp_pool.tile([P, 2 * JT], F32, tag="rs")

                def load_rs():
                    head_eng.dma_start(rs_t[:], rs_d[:, :])

                if not c0_first:
                    load_rs()
                xf = []          # per tile j: (chunk_tile, index within chunk)
                j0 = 0
                for ci, csz in enumerate(CHUNKS):
                    t = xf_pool.tile([P, csz, C], F16, name=f"xfc{ci}", tag="xf")
                    eng = nc.scalar if (ring_alt and ci % 2) else nc.sync
                    if swdge_head and ci == 0:
                        eng = nc.gpsimd
                    eng.dma_start(t[:], x_d[:, j0 : j0 + csz, :])
                    for jj in range(csz):
                        xf.append((t, jj))
                    j0 += csz
                    if c0_first and ci == 0:
                        load_rs()

                acc = [
                    psum_pool.tile([P, C], F32, name=f"acc{m}", tag=f"acc{m}")
                    for m in range(MC)
                ]

                # HAM warm-up: dummy matmuls on the iota constant keep the PE
                # busy through its ~3.4us cold window while the x stream
                # starts up, so the real matmuls run at 2.4 GHz
                if warm:
                    dacc = psum_pool.tile([P, warm_n], F32, name="dacc", tag="dacc")
                    for k in range(warm):
                        nc.tensor.matmul(
                            dacc[:],
                            lhsT=iota16[:, 0:P],
                            rhs=iota16[:, 0:warm_n],
                            start=(k == 0),
                            stop=(k == warm - 1),
                            skip_group_check=True,
                        )

                H = C // 2
                if tail_split:
                    # window-3 accumulates channel halves into two PSUM banks
                    # so the first half's copy+store overlaps the second
                    # half's last matmul
                    acc3b = psum_pool.tile([P, H], F32, name="acc3b", tag="acc3b")

                seen = set()
                for j, m in mms:
                    seen.add(m)
                    oh = oh_pool.tile([P, P], F16, name=f"oh{j}_{m}", tag="oh")
                    nc.vector.tensor_scalar(
                        out=oh[:],
                        in0=iota16[:, m * P : (m + 1) * P],
                        scalar1=rs_t[:, j : j + 1],
                        scalar2=rs_t[:, JT + j : JT + j + 1],
                        op0=mybir.AluOpType.is_equal,
                        op1=mybir.AluOpType.mult,
                    )
                    xt, jj = xf[j]
                    split = tail_split and m == last_window
                    if split:
                        nc.tensor.matmul(
                            acc[m][:, :H],
                            lhsT=oh[:],
                            rhs=xt[:, jj, :H],
                            start=((j, m) == first[m]),
                            stop=((j, m) == last[m]),
                            skip_group_check=True,
                        )
                        nc.tensor.matmul(
                            acc3b[:],
                            lhsT=oh[:],
                            rhs=xt[:, jj, H:],
                            start=((j, m) == first[m]),
                            stop=((j, m) == last[m]),
                            skip_group_check=True,
                        )
                    else:
                        nc.tensor.matmul(
                            acc[m][:],
                            lhsT=oh[:],
                            rhs=xt[:, jj, :],
                            start=((j, m) == first[m]),
                            stop=((j, m) == last[m]),
                            skip_group_check=True,
                        )
                    if (j, m) == last[m]:
                        osb = out_pool.tile([P, C], F16, name=f"osb{m}", tag=f"osb{m}")
                        if m == last_window:
                            # split the tail copy across ACT+DVE (parallel),
                            # then one store (two serialize on HWDGE gen)
                            src_b = acc3b[:] if split else acc[m][:, H:]
                            nc.scalar.copy(osb[:, :H], acc[m][:, :H])
                            nc.vector.tensor_copy(osb[:, H:], src_b)
                            if one_store:
                                nc.scalar.dma_start(
                                    o_d[m * P : (m + 1) * P, :], osb[:]
                                )
                            else:
                                nc.scalar.dma_start(
                                    o_d[m * P : (m + 1) * P, :H], osb[:, :H]
                                )
                                nc.sync.dma_start(
                                    o_d[m * P : (m + 1) * P, H:], osb[:, H:]
                                )
                        else:
                            nc.scalar.copy(osb[:], acc[m][:])
                            nc.scalar.dma_start(o_d[m * P : (m + 1) * P, :], osb[:])

                for m in range(MC):
                    if m not in seen:
                        osb = out_pool.tile([P, C], F16, name=f"osbz{m}", tag=f"osb{m}")
                        nc.vector.memset(osb[:], 0.0)
                        nc.scalar.dma_start(o_d[m * P : (m + 1) * P, :], osb[:])

            HINTS = {
                "pe": (mybir.EngineType.PE,),
                "none": (),
                "all": (
                    mybir.EngineType.PE,
                    mybir.EngineType.SP,
                    mybir.EngineType.Activation,
                    mybir.EngineType.DVE,
                    mybir.EngineType.Pool,
                ),
            }
            if repeats == 1:
                body()
            else:
                with tc.For_i(0, repeats, 1, hint_engines=HINTS[hints]):
                    body()

    nc.compile()
    return nc


def _get_nc(wsets, **cfg):
    cfg = {**DEFAULT_CFG, **cfg}
    key = (tuple(wsets), tuple(sorted(cfg.items())))
    if key not in _CACHE:
        _CACHE[key] = _build(tuple(wsets), **cfg)
    return _CACHE[key]


def prepare(x, regions):
    """Sort/scale on host. Returns (wsets, in_maps)."""
    x = np.asarray(x, dtype=np.float32)
    regions = np.asarray(regions).astype(np.int64)
    B = x.shape[0]

    in_maps = []
    lo = np.full(JT, MC - 1, dtype=np.int64)
    hi = np.zeros(JT, dtype=np.int64)
    for b in range(B):
        order = np.argsort(regions[b], kind="stable")
        rs = regions[b][order]
        xs = x[b][order].astype(np.float16)
        cnt = np.bincount(rs, minlength=NR + 1)  # index = region id
        s_tok = (1.0 / cnt[rs]).astype(np.float32)
        r_tok = rs.astype(np.float32)

        # window of each sorted token; per-tile min/max window, union over cores
        w_tok = (rs - 1) // P
        wt = w_tok.reshape(JT, P)
        lo = np.minimum(lo, wt.min(axis=1))
        hi = np.maximum(hi, wt.max(axis=1))

        # device layout: [partition, tile, ...] with tile j = tokens [128j, 128j+128)
        xdev = np.ascontiguousarray(xs.reshape(JT, P, C).transpose(1, 0, 2))
        rdev = r_tok.reshape(JT, P).T
        sdev = s_tok.reshape(JT, P).T
        rsdev = np.ascontiguousarray(np.concatenate([rdev, sdev], axis=1))
        in_maps.append({"xall": xdev, "rs": rsdev})

    wsets = tuple((int(a), int(b)) for a, b in zip(lo, hi))
    return wsets, in_maps


def kernel(x, regions, max_n, _trace=False, _tmpdir=None, _cfg=None):
    x = np.asarray(x, dtype=np.float32)
    regions = np.asarray(regions)
    assert x.shape == (NCORES, T, C), x.shape
    assert regions.shape == (NCORES, T), regions.shape
    assert int(np.asarray(max_n)) == NR

    wsets, in_maps = prepare(x, regions)
    cfg = dict(_cfg or {})
    nc = _get_nc(wsets, **cfg)
    try:
        res = run_bass_kernel_spmd(
            nc, in_maps, core_ids=list(range(NCORES)), trace=_trace, tmpdir=_tmpdir
        )
    except Exception:
        # one retry for transient runtime/tunnel failures
        res = run_bass_kernel_spmd(
            nc, in_maps, core_ids=list(range(NCORES)), trace=_trace, tmpdir=_tmpdir
        )
    out = np.stack(
        [res.results[b]["out"].astype(np.float32) for b in range(NCORES)], axis=0
    )
    if _trace:
        kernel._last_results = res
    return out
